# revision 11
# baseline (speedup 1.0000x reference)
"""GNN encoder (ECCConv -> GATConv -> GlobalAvgPool -> Dense) on 8 trn2 NeuronCores.

Edge-parallel by destination node with FREE node->window assignment: each core
owns ~N/8 nodes, grouped into NWIN windows of <=128 nodes packed so every
window has <=256 incoming edges (exactly 2 edge-tiles/window, uniform across
cores -> one SPMD program).

ECC phase avoids all on-chip transposes via the Y-reformulation:
    Y^T[ki, d] = sum_e z[e, ki] * S[e, d]     (z = h (x) x outer product, S = one-hot dst)
computed as PE matmuls with z (natural edge-major layout) as lhsT and the
one-hot S as rhs, accumulated per window in PSUM; then
    agg^T[o, d] = W1r^T Y^T + root^T x^T      (per window, 9 matmuls)
which lands x1 directly in the transposed layout the GAT stage needs.
S is generated on-chip (iota/is_equal at DVE 4x). z runs at DVE 2x via a
doubled-h layout (packed [1,2] last dim on every operand). Per-node GAT rows
are exchanged with one AllGather; a_self[dst] comes from a second pair-gather
of the T table. Softmax finalize is batched across all windows.
"""
import sys

for _p in ("/opt/trn_rl_repo", "/root/.axon_site/_ro/trn_rl_repo"):
    if _p not in sys.path:
        sys.path.append(_p)

import numpy as np
import ml_dtypes

import concourse.bass as bass
import concourse.bacc as bacc
import concourse.tile as tile
import concourse.mybir as mybir
import concourse.bass_utils as bass_utils
from concourse.masks import make_identity
from concourse.library_config import mlp

F32 = mybir.dt.float32
BF16 = mybir.dt.bfloat16
I16 = mybir.dt.int16
I8 = mybir.dt.int8
BF = ml_dtypes.bfloat16

N = 50000
E = 100000
F_IN = 32
F_E = 8
F1 = 64
F2 = 64
KH = 32
FC = 32
NCORES = 8
WIN = 128
TPW = 2                      # edge tiles per window (uniform)
ST = 8                       # edge tiles per super-tile (8*128 idx = SWDGE ring)
TW = 67                      # T row prefix [xp(64) | 1 | a_neigh | a_self]
WM = 66                      # weighted-message width [xp(64) | denom | junk]

_CACHE = {}


def _wrap_idx(a):
    """int16 index layout for dma_gather: [i%16, i//16], replicated to 128 rows."""
    ni = len(a)
    return np.tile(a.astype(np.int16).reshape(ni // 16, 16).T, (8, 1))


def _host_shard(edge_index):
    src = np.asarray(edge_index[0], np.int64)
    dst = np.asarray(edge_index[1], np.int64)
    deg = np.bincount(dst, minlength=N)

    # ---- nodes -> cores: greedy balance by in-degree (edge count) ----
    order = np.argsort(-deg, kind="stable")
    nwin = -(-((N // NCORES + WIN - 1) // WIN + 2) // 4) * 4   # 52 for N=50000
    while True:
        node_cap = nwin * WIN
        edge_cap = nwin * TPW * WIN
        core_e = np.zeros(NCORES, np.int64)
        core_n = np.zeros(NCORES, np.int64)
        node_core = np.full(N, -1, np.int64)
        ok = True
        for n in order:
            cands = [c for c in range(NCORES)
                     if core_n[c] < node_cap and core_e[c] + deg[n] <= edge_cap]
            if not cands:
                ok = False
                break
            c = min(cands, key=lambda c: (core_e[c], core_n[c]))
            node_core[n] = c
            core_e[c] += deg[n]
            core_n[c] += 1
        if ok:
            # ---- per-core: nodes -> windows (FFD, caps 128 nodes/256 edges) ----
            node_w = np.full(N, -1, np.int64)
            node_col = np.full(N, -1, np.int64)
            for c in range(NCORES):
                nodes = order[node_core[order] == c]       # degree-sorted
                we = np.zeros(nwin, np.int64)
                wn = np.zeros(nwin, np.int64)
                for n in nodes:
                    cands = np.nonzero((wn < WIN) & (we + deg[n] <= TPW * WIN))[0]
                    if len(cands) == 0:
                        ok = False
                        break
                    w = cands[np.argmin(we[cands])]
                    node_w[n] = w
                    node_col[n] = wn[w]
                    we[w] += deg[n]
                    wn[w] += 1
                if not ok:
                    break
        if ok:
            break
        nwin += 4

    npc_pad = nwin * WIN
    ntiles = nwin * TPW
    e_pad = ntiles * WIN
    srcT_all = node_core * npc_pad + node_w * WIN + node_col   # global T row/node

    # ---- per-core edge slotting: window w -> slots [w*256, w*256+cnt) ----
    e_core = node_core[dst]
    e_w = node_w[dst]
    eid = np.full((NCORES, e_pad), -1, np.int64)
    for c in range(NCORES):
        ids = np.nonzero(e_core == c)[0]
        ids = ids[np.argsort(e_w[ids], kind="stable")]
        w_of = e_w[ids]
        for w in range(nwin):
            wi = ids[w_of == w]
            eid[c, w * TPW * WIN: w * TPW * WIN + len(wi)] = wi
    return eid, node_col, srcT_all, nwin, ntiles, e_pad, deg


def _host_inputs(inputs):
    x = np.asarray(inputs["x"], np.float32)
    e = np.asarray(inputs["e"], np.float32)
    eid, node_col, srcT_all, nwin, ntiles, e_pad, deg = _host_shard(
        inputs["edge_index"])
    src = np.asarray(inputs["edge_index"][0], np.int64)
    dst = np.asarray(inputs["edge_index"][1], np.int64)
    npc_pad = nwin * WIN

    w0 = np.asarray(inputs["ecc_w0"], np.float32)
    b0 = np.asarray(inputs["ecc_b0"], np.float32)
    w1 = np.asarray(inputs["ecc_w1"], np.float32)
    b1 = np.asarray(inputs["ecc_b1"], np.float32)
    root = np.asarray(inputs["ecc_root"], np.float32)
    ecc_bias = np.asarray(inputs["ecc_bias"], np.float32)
    gk = np.asarray(inputs["gat_kernel"], np.float32)
    a_s = np.asarray(inputs["gat_attn_self"], np.float32)
    a_n = np.asarray(inputs["gat_attn_neigh"], np.float32)
    gat_bias = np.asarray(inputs["gat_bias"], np.float32)
    fc_w = np.asarray(inputs["fc_w"], np.float32)
    fc_b = np.asarray(inputs["fc_b"], np.float32)

    use_b0 = bool(np.any(b0))
    use_b1 = bool(np.any(b1))
    use_gbias = bool(np.any(gat_bias))
    ke = F_E + 1 if use_b0 else F_E
    nchunk = 9 if use_b1 else 8

    w0m = np.vstack([w0, b0[None, :]]) if use_b0 else w0
    W1r = w1.reshape(KH, F_IN, F1).reshape(KH * F_IN, F1)
    W1re = np.concatenate([W1r[128 * b: 128 * (b + 1)] for b in range(8)], axis=1)
    W1_9 = b1.reshape(F_IN, F1)
    root_ext = np.vstack([root, ecc_bias[None, :]])
    attn2 = np.stack([a_n, a_s], axis=1)               # a_ps row0=a_neigh, row1=a_self
    ident_tbl = np.zeros((256, 128), BF)
    ident_tbl[:128, :128] = np.eye(128, dtype=BF)
    shared = {
        "ident_tbl": ident_tbl,
        "w0m": np.ascontiguousarray(w0m.astype(BF)),
        "w0d": np.ascontiguousarray(np.repeat(w0m, 2, axis=1).astype(BF)),
        "W1re": np.ascontiguousarray(W1re.astype(BF)),
        "W1f8": np.ascontiguousarray((W1re * 16.0).astype(
            ml_dtypes.float8_e4m3)),
        "W1_9": np.ascontiguousarray(W1_9.astype(BF)),
        "root_ext": np.ascontiguousarray(root_ext.astype(BF)),
        "gk": np.ascontiguousarray(gk.astype(BF)),
        "attn2": np.ascontiguousarray(attn2.astype(BF)),
        "gbias": np.ascontiguousarray(np.tile(gat_bias[None, :].astype(np.float32),
                                              (128, 1))),
        "fc_w": np.ascontiguousarray(fc_w),
        "fc_b": np.ascontiguousarray(fc_b.reshape(FC, 1)),
    }

    x128 = np.zeros((N, 128), BF)
    x128[:, :F_IN] = x.astype(BF)

    per_core = []
    u_max = 0
    for c in range(NCORES):
        ids = eid[c]
        valid = ids >= 0
        idsv = np.where(valid, ids, 0)
        s_glob = np.where(valid, src[idsv], 0)
        uniq, inv = np.unique(s_glob, return_inverse=True)
        u_max = max(u_max, len(uniq))
        per_core.append((ids, valid, idsv, s_glob, uniq, inv))
    u_pad = int(np.ceil(u_max / 128) * 128)

    in_maps = []
    for c in range(NCORES):
        ids, valid, idsv, s_glob, uniq, inv = per_core[c]

        e_T = np.where(valid[None, :], e[idsv].T, 0.0)
        if use_b0:
            e_T = np.vstack([e_T, valid[None, :].astype(np.float32)])

        x_c = np.zeros((u_pad, 128), BF)
        x_c[:len(uniq)] = x128[uniq]
        xg_idx = _wrap_idx(inv)

        srcT = srcT_all[s_glob]
        t2_idx = _wrap_idx(srcT // 2)
        par = (srcT % 2).astype(np.int8).reshape(ntiles, 128).T

        dstT = srcT_all[np.where(valid, dst[idsv], 0)]
        t2d_idx = _wrap_idx(dstT // 2)
        pard = (dstT % 2).astype(np.int8).reshape(ntiles, 128).T

        col = np.where(valid, node_col[dst[idsv]], 128)
        colv = col.astype(np.float32).reshape(ntiles, 128).T   # [128, ntiles]
        scol_idx = _wrap_idx(col)

        # x^T per local node layout [F_IN+1, npc_pad]; ones row marks real nodes
        x_T = np.zeros((F_IN + 1, npc_pad), np.float32)
        pool_mask = np.zeros((128, nwin), np.float32)
        mine = np.nonzero((srcT_all // npc_pad) == c)[0]
        loc = srcT_all[mine] - c * npc_pad
        x_T[:F_IN, loc] = x[mine].T
        x_T[F_IN, loc] = 1.0
        pool_mask[loc % WIN, loc // WIN] = 1.0

        m = {
            "e_T": np.ascontiguousarray(e_T.astype(BF)),
            "x_c": x_c,
            "xg_idx": np.ascontiguousarray(xg_idx),
            "t2_idx": np.ascontiguousarray(t2_idx),
            "t2d_idx": np.ascontiguousarray(t2d_idx),
            "par": np.ascontiguousarray(par),
            "pard": np.ascontiguousarray(pard),
            "colv": np.ascontiguousarray(colv),
            "scol_idx": np.ascontiguousarray(scol_idx),
            "x_T": np.ascontiguousarray(x_T.astype(BF)),
            "pool_mask": np.ascontiguousarray(pool_mask.astype(BF)),
        }
        m.update(shared)
        in_maps.append(m)

    meta = dict(ke=ke, nchunk=nchunk, nwin=nwin, ntiles=ntiles, e_pad=e_pad,
                u_pad=u_pad, use_gbias=use_gbias)
    return in_maps, meta


def build_nc(meta, use_collectives=True, num_devices=NCORES, debug_dump=False):
    ke, nchunk = meta["ke"], meta["nchunk"]
    nwin, ntiles, e_pad, u_pad = (meta["nwin"], meta["ntiles"], meta["e_pad"],
                                  meta["u_pad"])
    use_gbias = meta["use_gbias"]
    npc_pad = nwin * WIN
    nsup = ntiles // ST
    assert ntiles % ST == 0 and ST % TPW == 0

    nc = bacc.Bacc("TRN2", target_bir_lowering=False, debug=False,
                   enable_asserts=False, num_devices=num_devices)

    def din(name, shape, dt=F32):
        return nc.dram_tensor(name, shape, dt, kind="ExternalInput").ap()

    e_T = din("e_T", [ke, e_pad], BF16)
    x_c = din("x_c", [u_pad, 128], BF16)
    xg_idx = din("xg_idx", [128, e_pad // 16], I16)
    t2_idx = din("t2_idx", [128, e_pad // 16], I16)
    t2d_idx = din("t2d_idx", [128, e_pad // 16], I16)
    par_d = din("par", [128, ntiles], I8)
    pard_d = din("pard", [128, ntiles], I8)
    colv_d = din("colv", [128, ntiles])
    scol_idx = din("scol_idx", [128, e_pad // 16], I16)
    ident_tbl = din("ident_tbl", [256, 128], BF16)
    x_T = din("x_T", [F_IN + 1, npc_pad], BF16)
    pool_mask = din("pool_mask", [128, nwin], BF16)
    w0m = din("w0m", [ke, KH], BF16)
    w0d = din("w0d", [ke, KH * 2], BF16)
    W1re = din("W1re", [128, F1 * 8], BF16)
    W1f8 = din("W1f8", [128, F1 * 8], mybir.dt.float8e4)
    W1_9 = din("W1_9", [F_IN, F1], BF16)
    root_ext = din("root_ext", [F_IN + 1, F1], BF16)
    gk = din("gk", [F2, F2], BF16)
    attn2 = din("attn2", [F2, 2], BF16)
    gbias = din("gbias", [128, F2])
    fc_w = din("fc_w", [F2, FC])
    fc_b = din("fc_b", [FC, 1])
    out_d = nc.dram_tensor("out", [FC, 1], F32, kind="ExternalOutput").ap()
    if debug_dump:
        y_dbg = nc.dram_tensor("y_dbg", [128, 1024 * nwin], F32,
                               kind="ExternalOutput").ap()
        x1_dbg = nc.dram_tensor("x1_dbg", [F1, npc_pad], F32,
                                kind="ExternalOutput").ap()

    with tile.TileContext(nc) as tc:
        nc.gpsimd.load_library(mlp)
        with (
            tc.tile_pool(name="res", bufs=1) as res,
            tc.tile_pool(name="dram", bufs=1, space="DRAM") as drp,
        ):
            # ---------- preamble loads ----------
            xgi_sb = res.tile([128, e_pad // 16], I16)
            nc.sync.dma_start(xgi_sb[:], xg_idx[:])
            colv_sb = res.tile([128, ntiles], F32)
            nc.sync.dma_start(colv_sb[:], colv_d[:])
            w0d_sb = res.tile([ke, KH * 2], BF16)
            nc.sync.dma_start(w0d_sb[:], w0d[:])
            W1f8_sb = res.tile([128, F1 * 8], mybir.dt.float8e4)
            nc.scalar.dma_start(W1f8_sb[:], W1f8[:])
            root_sb = res.tile([F_IN + 1, F1], BF16)
            nc.scalar.dma_start(root_sb[:], root_ext[:])
            t2i_sb = res.tile([128, e_pad // 16], I16)
            nc.scalar.dma_start(t2i_sb[:], t2_idx[:])
            par_sb = res.tile([128, ntiles], I8)
            nc.scalar.dma_start(par_sb[:], par_d[:])
            gk_sb = res.tile([F2, F2], BF16)
            nc.scalar.dma_start(gk_sb[:], gk[:])
            attn_sb = res.tile([F2, 2], BF16)
            nc.scalar.dma_start(attn_sb[:], attn2[:])
            fcw_sb = res.tile([F2, FC], F32)
            nc.scalar.dma_start(fcw_sb[:], fc_w[:])
            fcb_sb = res.tile([FC, 1], F32)
            nc.scalar.dma_start(fcb_sb[:], fc_b[:])
            W1_sb = res.tile([128, F1 * 8], BF16)
            nc.scalar.dma_start(W1_sb[:], W1re[:])
            W19_sb = res.tile([F_IN, F1], BF16)
            nc.scalar.dma_start(W19_sb[:], W1_9[:])
            xT_sb = res.tile([F_IN + 1, npc_pad], BF16)
            nc.scalar.dma_start(xT_sb[:], x_T[:])
            if use_gbias:
                gbias_sb = res.tile([128, F2], F32)
                nc.sync.dma_start(gbias_sb[:], gbias[:])
                pmask_sb = res.tile([128, nwin], BF16)
                nc.sync.dma_start(pmask_sb[:], pool_mask[:])
            ident_bf = res.tile([128, 128], BF16)
            make_identity(nc, ident_bf[:])
            iota_i = res.tile([128, 128], I16)
            nc.gpsimd.iota(iota_i[:], pattern=[[1, 128]], channel_multiplier=0)
            iota_bf = res.tile([128, 128], BF16)
            nc.vector.tensor_copy(iota_bf[:], iota_i[:])

            s_all = res.tile([128, ntiles * 128], BF16)
            x1T_all = res.tile([F1, nwin * WIN], BF16)
            Tt_all = res.tile([128, nwin * TW], BF16)
            nc.vector.memset(
                Tt_all[:].rearrange("p (w f) -> p w f", f=TW)[:, :, F2:F2 + 1],
                1.0)
            o2_all = res.tile([128, nwin * WM], BF16)
            x2_all = res.tile([128, F2 * nwin], BF16)

            T_loc = drp.tile([npc_pad, 128], BF16)
            T_full = drp.tile([NCORES * npc_pad, 128], BF16)
            pool_in = drp.tile([F2, 1], F32)
            pool_out = drp.tile([F2, 1], F32)

            # ============ Phase A: ECC ============
            WB = 4
            with (
                tc.tile_pool(name="a_sb", bufs=4) as sa,
                tc.tile_pool(name="a_z", bufs=6) as sz,
                tc.tile_pool(name="a_yt", bufs=2) as syt,
                tc.tile_pool(name="b_sb", bufs=3) as sb,
                tc.tile_pool(name="a_h", bufs=1, space="PSUM") as ph,
                tc.tile_pool(name="a_ytp", bufs=2, space="PSUM") as pyt,
                tc.tile_pool(name="a_agg", bufs=1, space="PSUM") as pag,
                tc.tile_pool(name="b_ps", bufs=1, space="PSUM") as pb,
            ):

                def emit_a2(g):
                    w0_ = g * WB
                    xsl = x1T_all[:, w0_ * WIN:(w0_ + WB) * WIN]
                    xpt_ps = pb.tile([F2, WB * WIN], F32, space="PSUM",
                                     tag="xpt", name=f"xpt_{g}")
                    nc.tensor.matmul(out=xpt_ps[:], lhsT=gk_sb[:], rhs=xsl,
                                     start=True, stop=True,
                                     skip_group_check=True)
                    xpt_sb = sb.tile([F2, WB * WIN], BF16, tag="xpt_sb")
                    nc.scalar.activation(xpt_sb[:], xpt_ps[:],
                                         mybir.ActivationFunctionType.Copy)
                    a_ps = pb.tile([2, WB * WIN], F32, space="PSUM", tag="xpt",
                                   name=f"a_{g}")
                    nc.tensor.matmul(out=a_ps[:], lhsT=attn_sb[:], rhs=xpt_sb[:],
                                     start=True, stop=True,
                                     skip_group_check=True)
                    a_sb = sb.tile([2, WB * WIN], BF16, tag="a_sb")
                    nc.vector.tensor_copy(a_sb[:], a_ps[:])
                    xpac = pb.tile([128, WB * (F2 + 2)], BF16, space="PSUM",
                                   tag="xpac", name=f"xpac_{g}")
                    xp_ps = xpac[:, :WB * F2]
                    ac_ps = xpac[:, WB * F2:]
                    for ws in range(WB):
                        nc.tensor.transpose(
                            out=xp_ps[:, ws * F2:(ws + 1) * F2],
                            in_=xpt_sb[:, ws * WIN:(ws + 1) * WIN],
                            identity=ident_bf[:F2, :F2])
                        nc.tensor.transpose(
                            out=ac_ps[:, ws * 2:(ws + 1) * 2],
                            in_=a_sb[:, ws * WIN:(ws + 1) * WIN],
                            identity=ident_bf[:2, :2])
                    nc.scalar.activation(
                        Tt_all[:, w0_ * TW:(w0_ + WB) * TW].rearrange(
                            "p (w f) -> p w f", f=TW)[:, :, :F2],
                        xp_ps.rearrange("p (w f) -> p w f", f=F2),
                        mybir.ActivationFunctionType.Copy)
                    nc.vector.tensor_copy(
                        Tt_all[:, w0_ * TW:(w0_ + WB) * TW].rearrange(
                            "p (w f) -> p w f", f=TW)[:, :, F2 + 1:F2 + 3],
                        ac_ps.rearrange("p (w f) -> p w f", f=2))

                def emit_twrite(wlo, whi):
                    nc.sync.dma_start(
                        T_loc[wlo * WIN:whi * WIN, :TW].rearrange(
                            "(w p) f -> p w f", p=WIN),
                        Tt_all[:, wlo * TW:whi * TW].rearrange(
                            "p (w f) -> p w f", f=TW))
                for s in range(nsup):
                    t0 = s * ST
                    eT_t = sa.tile([ke, ST * 128], BF16, tag="eT")
                    nc.sync.dma_start(eT_t[:], e_T[:, t0 * 128:(t0 + ST) * 128])
                    xg = sa.tile([128, ST * 128], BF16, tag="xg")
                    nc.gpsimd.dma_gather(
                        out_ap=xg[:].rearrange("p (c e) -> p c e", e=128),
                        in_ap=x_c[:],
                        idxs_ap=xgi_sb[:, s * (ST * 8):(s + 1) * (ST * 8)],
                        num_idxs=ST * 128, num_idxs_reg=ST * 128, elem_size=128)
                    for j in range(ST):
                        t = t0 + j
                        nc.vector.tensor_scalar(
                            out=s_all[:, t * 128:(t + 1) * 128],
                            in0=iota_bf[:], scalar1=colv_sb[:, t:t + 1],
                            scalar2=None, op0=mybir.AluOpType.is_equal)

                    h2_ps = ph.tile([128, ST * KH * 2], F32, space="PSUM",
                                    tag="h")
                    for j in range(ST):
                        nc.tensor.matmul(out=h2_ps[:, j * 64:(j + 1) * 64],
                                         lhsT=eT_t[:, j * 128:(j + 1) * 128],
                                         rhs=w0d_sb[:],
                                         start=(j % 8 == 0), stop=True,
                                         skip_group_check=True)
                    h2 = sa.tile([128, ST * KH * 2], BF16, tag="h2")
                    nc.vector.tensor_scalar(out=h2[:], in0=h2_ps[:],
                                            scalar1=0.0, scalar2=None,
                                            op0=mybir.AluOpType.max)

                    for j in range(ST):
                        t = t0 + j
                        w = t // TPW
                        z_t = sz.tile([128, KH * F_IN], BF16, tag="z")
                        zv = z_t[:].rearrange("p (k ih il) -> p k ih il",
                                              k=KH, ih=F_IN // 2)
                        zeng = nc.gpsimd if j % 4 == 3 else nc.vector
                        zeng.tensor_tensor(
                            out=zv,
                            in0=h2[:, j * 64:(j + 1) * 64]
                                .rearrange("p (k il) -> p k il", il=2)
                                .unsqueeze(2)
                                .broadcast_to([128, KH, F_IN // 2, 2]),
                            in1=xg[:, j * 128:j * 128 + F_IN]
                                .rearrange("p (ih il) -> p ih il", il=2)
                                .unsqueeze(1)
                                .broadcast_to([128, KH, F_IN // 2, 2]),
                            op=mybir.AluOpType.mult)
                        first = (t % TPW == 0)
                        last = (t % TPW == TPW - 1)
                        if first:
                            yt_ps = pyt.tile([128, 1024], F32, space="PSUM",
                                             tag="yt", name=f"yt_{w}")
                            if nchunk == 9:
                                yt9_ps = pag.tile([F_IN, 128], F32, space="PSUM",
                                                  tag="yt9", name=f"yt9_{w}")
                        for b in range(8):
                            # start only on the first chunk of each 2KB PSUM
                            # zero-region (start marks the WHOLE region pending)
                            nc.tensor.matmul(
                                out=yt_ps[:, b * 128:(b + 1) * 128],
                                lhsT=z_t[:, b * 128:(b + 1) * 128],
                                rhs=s_all[:, t * 128:(t + 1) * 128],
                                start=first and b % 4 == 0, stop=last,
                                skip_group_check=True)
                        if nchunk == 9:
                            nc.tensor.matmul(
                                out=yt9_ps[:], lhsT=xg[:, j * 128:j * 128 + F_IN],
                                rhs=s_all[:, t * 128:(t + 1) * 128],
                                start=first, stop=last, skip_group_check=True)
                        if not last:
                            continue

                        # ---- window drain ----
                        yt_sb = syt.tile([128, 1024], mybir.dt.float8e4,
                                         tag="yt_sb")
                        nc.scalar.activation(yt_sb[:], yt_ps[:],
                                             mybir.ActivationFunctionType.Copy,
                                             scale=1.0 / 16.0)
                        if w % 4 == 0:
                            aggp_ps = pag.tile([F1, 512], F32, space="PSUM",
                                               tag="agg", name=f"agg_{w}")
                            nc.tensor.matmul(
                                out=aggp_ps[:], lhsT=root_sb[:],
                                rhs=xT_sb[:, w * WIN:(w + 4) * WIN],
                                start=True, stop=False, skip_group_check=True)
                        agg_ps = aggp_ps[:, (w % 4) * 128:(w % 4 + 1) * 128]
                        if nchunk == 9:
                            yt9_sb = syt.tile([F_IN, 128], BF16, tag="yt9_sb")
                            nc.vector.tensor_copy(yt9_sb[:], yt9_ps[:])
                            nc.tensor.matmul(out=agg_ps, lhsT=W19_sb[:],
                                             rhs=yt9_sb[:], start=False,
                                             stop=False, skip_group_check=True)
                        for b2 in range(4):
                            nc.tensor.matmul(
                                out=agg_ps,
                                lhsT=W1f8_sb[:, b2 * 2 * F1:(b2 + 1) * 2 * F1]
                                    .rearrange("p (two f) -> p two f", two=2),
                                rhs=yt_sb[:, b2 * 256:(b2 + 1) * 256]
                                    .rearrange("p (two f) -> p two f", two=2),
                                start=False, stop=(b2 == 3 and w % 4 == 3),
                                perf_mode=mybir.MatmulPerfMode.DoubleRow,
                                skip_group_check=True)
                        if w % 4 == 3:
                            if (w // 4) % 2 == 0:
                                nc.scalar.activation(
                                    x1T_all[:, (w - 3) * WIN:(w + 1) * WIN],
                                    aggp_ps[:],
                                    mybir.ActivationFunctionType.Relu)
                            else:
                                nc.vector.tensor_scalar(
                                    out=x1T_all[:, (w - 3) * WIN:(w + 1) * WIN],
                                    in0=aggp_ps[:], scalar1=0.0, scalar2=None,
                                    op0=mybir.AluOpType.max)
                        if w % WB == WB - 1:
                            emit_a2(w // WB)
                        if w % 16 == 15:
                            emit_twrite(w - 15, w + 1)
                        elif w == nwin - 1:
                            emit_twrite(nwin - nwin % 16, nwin)
                        if debug_dump:
                            ydf = sa.tile([128, 1024], F32, tag="ydf")
                            nc.vector.tensor_copy(ydf[:], yt_ps[:])
                            nc.sync.dma_start(
                                y_dbg[:, w * 1024:(w + 1) * 1024], ydf[:])
                            x1f = sa.tile([F1, 128], F32, tag="x1f")
                            nc.vector.tensor_copy(x1f[:], agg_ps[:])
                            nc.sync.dma_start(
                                x1_dbg[:, w * 128:(w + 1) * 128], x1f[:])


            # ============ AllGather T ============
            if use_collectives:
                nc.gpsimd.collective_compute(
                    "AllGather", mybir.AluOpType.bypass,
                    replica_groups=[list(range(NCORES))],
                    ins=[T_loc.opt()], outs=[T_full.opt()])
            else:
                nb4 = npc_pad // 4
                for g4 in range(4):
                    nc.sync.dma_start(T_full[g4 * nb4:(g4 + 1) * nb4, :TW],
                                      T_loc[g4 * nb4:(g4 + 1) * nb4, :TW])

            T2 = T_full[:].rearrange("(v two) f -> v (two f)", two=2)

            # ============ Phase C: GAT edges ============
            with (
                tc.tile_pool(name="c_sb", bufs=4) as sc,
                tc.tile_pool(name="c_o2", bufs=3, space="PSUM") as po2,
                tc.tile_pool(name="c_st", bufs=2, space="PSUM") as pst,
                tc.tile_pool(name="c_fin", bufs=1, space="PSUM") as pfin,
            ):
                pool_ps = pfin.tile([F2, 1], F32, space="PSUM", tag="pool")
                ones_sb = sc.tile([128, 1], F32, tag="ones")
                nc.vector.memset(ones_sb[:], 1.0)

                o2v = o2_all[:].rearrange("p (f w) -> p f w", w=nwin)
                x2v_full = x2_all[:].rearrange("p (f w) -> p f w", f=F2)

                def emit_fin(lo, hi, part):
                    nwh = hi - lo
                    dn = sc.tile([128, nwh], F32, tag=f"dn{part}")
                    nc.vector.tensor_scalar(
                        out=dn[:], in0=o2_all[:, F2 * nwin + lo:F2 * nwin + hi],
                        scalar1=1e-9, scalar2=None, op0=mybir.AluOpType.add)
                    rcp = sc.tile([128, nwh], F32, tag=f"rcp{part}")
                    nc.vector.reciprocal(rcp[:], dn[:])
                    x2h = x2v_full[:, :, lo:hi]
                    nc.vector.tensor_tensor(
                        out=x2h,
                        in0=o2v[:, :F2, lo:hi],
                        in1=rcp[:].unsqueeze(1).broadcast_to([128, F2, nwh]),
                        op=mybir.AluOpType.mult)
                    xrh = sc.tile([128, F2], F32, tag=f"xr{part}")
                    nc.vector.tensor_reduce(out=xrh[:].unsqueeze(2), in_=x2h,
                                            axis=mybir.AxisListType.X,
                                            op=mybir.AluOpType.add)
                    nc.tensor.matmul(out=pool_ps[:], lhsT=xrh[:], rhs=ones_sb[:],
                                     start=(part == 0), stop=(part == 1),
                                     skip_group_check=True)

                def emit_tg(s_):
                    Tg_ = sc.tile([128, ST * 256], BF16, tag="Tg")
                    nc.gpsimd.dma_gather(
                        out_ap=Tg_[:].rearrange("p (c e) -> p c e", e=256),
                        in_ap=T2,
                        idxs_ap=t2i_sb[:, s_ * (ST * 8):(s_ + 1) * (ST * 8)],
                        num_idxs=ST * 128, num_idxs_reg=ST * 128, elem_size=256)
                    return Tg_

                # quarter-finalize fire points: first pair-drain (odd w)
                # at or after each quarter boundary
                fin_fire = {}
                for q_ in range(3):
                    b_ = (q_ + 1) * (nwin // 4)
                    wq = b_ - 1 if (b_ - 1) % 2 == 1 else b_
                    fin_fire[wq] = q_
                # part-0 finalize fires at a quad-drain (w%4==3)
                fin_split = (nwin // 2 + 3) // 4 * 4
                assert nwin % 4 == 0
                tg_cur = emit_tg(0)
                for s in range(nsup):
                    t0 = s * ST
                    Tg = tg_cur
                    Tg3 = Tg[:].rearrange("p (c e) -> p c e", e=256)
                    # a_self[dst] via on-chip S^T: asd[e] = sum_d S^T[d,e]*aself[d]
                    st_ps = pst.tile([128, ST * 128], BF16, space="PSUM",
                                     tag="st", name=f"st_{s}")
                    for j in range(ST):
                        nc.tensor.transpose(
                            out=st_ps[:, j * 128:(j + 1) * 128],
                            in_=s_all[:, (t0 + j) * 128:(t0 + j + 1) * 128],
                            identity=ident_bf[:])
                    st_sb = sc.tile([128, ST * 128], BF16, tag="st_sb")
                    nc.scalar.activation(st_sb[:], st_ps[:],
                                         mybir.ActivationFunctionType.Copy)
                    asd_ps = pst.tile([128, ST], F32, space="PSUM", tag="asdp",
                                      name=f"asdp_{s}")
                    for j in range(ST):
                        w = (t0 + j) // TPW
                        nc.tensor.matmul(
                            out=asd_ps[:, j:j + 1],
                            lhsT=st_sb[:, j * 128:(j + 1) * 128],
                            rhs=Tt_all[:, w * TW + F2 + 2:w * TW + F2 + 3],
                            start=True, stop=True, skip_group_check=True)

                    selv_t = sc.tile([128, ST * (TW + 1)], BF16, tag="selv")
                    selv = selv_t[:].rearrange("p (t f) -> p t f",
                                               f=TW + 1)[:, :, :TW]
                    if s % 2 == 0:
                        nc.scalar.activation(selv, Tg3[:, :, :TW],
                                             mybir.ActivationFunctionType.Copy)
                    else:
                        nc.vector.tensor_copy(selv, Tg3[:, :, :TW])
                    nc.vector.copy_predicated(
                        selv,
                        par_sb[:, t0:t0 + ST].unsqueeze(2)
                            .broadcast_to([128, ST, TW]),
                        Tg3[:, :, 128:128 + TW])
                    sco = sc.tile([128, ST], F32, tag="sco")
                    nc.vector.tensor_tensor(out=sco[:].unsqueeze(2),
                                            in0=asd_ps[:].unsqueeze(2),
                                            in1=selv[:, :, F2 + 1:F2 + 2],
                                            op=mybir.AluOpType.add)
                    lr = sc.tile([128, ST], F32, tag="lr")
                    nc.vector.tensor_scalar(out=lr[:], in0=sco[:], scalar1=0.2,
                                            scalar2=None, op0=mybir.AluOpType.mult)
                    nc.vector.tensor_tensor(out=lr[:], in0=lr[:], in1=sco[:],
                                            op=mybir.AluOpType.max)
                    ex = sc.tile([128, ST], BF16, tag="ex")
                    nc.scalar.activation(ex[:], lr[:],
                                         mybir.ActivationFunctionType.Exp)
                    if s + 1 < nsup:
                        tg_cur = emit_tg(s + 1)
                    ex2 = sc.tile([128, ST * 2], BF16, tag="ex2")
                    nc.vector.tensor_copy(
                        ex2[:].rearrange("p (t two) -> p t two", two=2),
                        ex[:].unsqueeze(2).broadcast_to([128, ST, 2]))

                    # wm[p,t,0:64]=xp*ex, [64]=1*ex (denom), [65]=a_neigh*ex junk
                    wm = sc.tile([128, ST * WM], BF16, tag="wm")
                    nc.vector.tensor_tensor(
                        out=wm[:].rearrange("p (t fh il) -> p t fh il", t=ST,
                                            il=2),
                        in0=selv[:, :, :WM].rearrange("p t (fh il) -> p t fh il",
                                                      il=2),
                        in1=ex2[:].rearrange("p (t il) -> p t il", il=2)
                            .unsqueeze(2).broadcast_to([128, ST, WM // 2, 2]),
                        op=mybir.AluOpType.mult)
                    wm3 = wm[:].rearrange("p (t f) -> p t f", f=WM)

                    for j in range(ST):
                        t = t0 + j
                        w = t // TPW
                        first = (t % TPW == 0)
                        last = (t % TPW == TPW - 1)
                        if first and w % 4 == 0:
                            o2p_ps = po2.tile([128, 4 * WM], F32, space="PSUM",
                                              tag="o2", name=f"o2_{w}")
                        nc.tensor.matmul(out=o2p_ps[:, (w % 4) * WM:
                                                    (w % 4 + 1) * WM],
                                         lhsT=s_all[:, t * 128:(t + 1) * 128],
                                         rhs=wm3[:, j, :],
                                         start=(first and w % 4 == 0),
                                         stop=(last and w % 4 == 3),
                                         skip_group_check=True)
                        if last and w % 4 == 3:
                            # rcp>0 so relu commutes with the scaling (bias=0)
                            fn = (mybir.ActivationFunctionType.Copy if use_gbias
                                  else mybir.ActivationFunctionType.Relu)
                            nc.scalar.activation(
                                o2v[:, :, w - 3:w + 1].rearrange(
                                    "p f wl -> p wl f"),
                                o2p_ps[:].rearrange("p (wl f) -> p wl f", wl=4),
                                fn)
                            if not use_gbias and w == fin_split - 1:
                                emit_fin(0, fin_split, 0)

                # ---------- batched softmax finalize + pool ----------
                if use_gbias:
                    dn = sc.tile([128, nwin], F32, tag="dn")
                    nc.vector.tensor_scalar(
                        out=dn[:], in0=o2_all[:, F2 * nwin:(F2 + 1) * nwin],
                        scalar1=1e-9, scalar2=None, op0=mybir.AluOpType.add)
                    rcp = sc.tile([128, nwin], F32, tag="rcp")
                    nc.vector.reciprocal(rcp[:], dn[:])
                    nc.vector.tensor_tensor(
                        out=x2v_full,
                        in0=o2v[:, :F2, :],
                        in1=rcp[:].unsqueeze(1).broadcast_to([128, F2, nwin]),
                        op=mybir.AluOpType.mult)
                    nc.vector.tensor_tensor(
                        out=x2v_full, in0=x2v_full,
                        in1=gbias_sb[:].rearrange("p f -> p f 1")
                            .broadcast_to([128, F2, nwin]),
                        op=mybir.AluOpType.add)
                    nc.scalar.activation(x2_all[:], x2_all[:],
                                         mybir.ActivationFunctionType.Relu)
                    nc.vector.tensor_tensor(
                        out=x2v_full, in0=x2v_full,
                        in1=pmask_sb[:].unsqueeze(1).broadcast_to([128, F2, nwin]),
                        op=mybir.AluOpType.mult)
                    xr = sc.tile([128, F2], F32, tag="xr")
                    nc.vector.tensor_reduce(out=xr[:].unsqueeze(2), in_=x2v_full,
                                            axis=mybir.AxisListType.X,
                                            op=mybir.AluOpType.add)
                    nc.tensor.matmul(out=pool_ps[:], lhsT=xr[:], rhs=ones_sb[:],
                                     start=True, stop=True,
                                     skip_group_check=True)
                else:
                    emit_fin(fin_split, nwin, 1)

                pooled = sc.tile([F2, 1], F32, tag="pooled")
                nc.scalar.activation(pooled[:], pool_ps[:],
                                     mybir.ActivationFunctionType.Copy,
                                     scale=1.0 / N)
                nc.sync.dma_start(pool_in[:], pooled[:])
                if use_collectives:
                    nc.gpsimd.collective_compute(
                        "AllReduce", mybir.AluOpType.add,
                        replica_groups=[list(range(NCORES))],
                        ins=[pool_in.opt()], outs=[pool_out.opt()])
                else:
                    nc.sync.dma_start(pool_out[:], pool_in[:])
                pooled2 = sc.tile([F2, 1], F32, tag="pooled2")
                nc.sync.dma_start(pooled2[:], pool_out[:])
                fc_ps = pfin.tile([FC, 1], F32, space="PSUM", tag="pool")
                nc.tensor.matmul(out=fc_ps[:], lhsT=fcw_sb[:], rhs=pooled2[:],
                                 start=True, stop=True)
                out_sb = sc.tile([FC, 1], F32, tag="out")
                nc.scalar.activation(out_sb[:], fc_ps[:],
                                     mybir.ActivationFunctionType.Relu,
                                     bias=fcb_sb[:, :1])
                nc.sync.dma_start(out_d[:], out_sb[:])

    nc.compile()
    return nc


def kernel(**inputs):
    in_maps, meta = _host_inputs(inputs)
    key = tuple(sorted(meta.items()))
    if key not in _CACHE:
        _CACHE[key] = build_nc(meta)
    nc = _CACHE[key]
    res = bass_utils.run_bass_kernel_spmd(nc, in_maps, core_ids=list(range(NCORES)))
    return res.results[0]["out"].reshape(FC).astype(np.float32)
